# revision 17
# baseline (speedup 1.0000x reference)
"""Causal self-attention with RoPE on 8 TRN2 NeuronCores.

Sharding: tensor-parallel over heads (H=8 -> 1 head per core).
v2: per-block proj/attn software pipeline (keeps PE warm), merged FD=1024
exp ops on ACT, rope pair-swap via DVE stream_shuffle (instead of a second
PE projection), bf16 cos/sin tables, merged v-cast and yt+denominator cast.

Each core computes, for its head h:
    q,k projections (bf16 matmuls, fp32 PSUM) -> pair-swap via stream_shuffle
    -> RoPE on DVE (bf16 tables)  -> v projection
    S^T blocks (j,i) via K=64 row-paired concurrent matmuls into 2-bank tiles
    P^T = exp(S^T/8) on ACT (bf16 out, FD=1024 for full pairs), causal
    masking via gpsimd affine_select (covers prefix + diagonal band)
    y_u^T = [v | ones]^T-weighted PV matmuls  (row 64 = softmax denominator)
    out_u = y_u @ Wp_h^T on-device; host computes sum_h out_u_h / colsum_h.
"""
import sys

sys.path.insert(0, "/opt/trn_rl_repo")

import numpy as np
import ml_dtypes

import concourse.bass as bass
import concourse.mybir as mybir
import concourse.tile as tile
from concourse.bass_utils import run_bass_kernel_spmd

B, T, C, H = 1, 4096, 512, 8
HS = C // H  # 64
NCORES = 8
TB = 512           # t-block width for projections / i-block width for attention
NTB = T // TB      # 8
JC = 128           # j-chunk width
NJC = T // JC      # 32

_ctr = [0]


def _legalize_waits(nc):
    """This walrus build accepts at most one sem-wait command per hw
    instruction; move extra waits onto same-engine NoOps inserted before."""
    for f in nc.m.functions:
        for bb in f.blocks:
            insts = bb.instructions
            out = []
            for inst in insts:
                si = inst.sync_info
                if si is not None and len(si.on_wait) > 1:
                    waits = list(si.on_wait)
                    for w in waits[:-1]:
                        _ctr[0] += 1
                        nop = mybir.InstNoOp(name=f"I-waitsplit-{_ctr[0]}")
                        nop.engine = inst.engine
                        nop.sync_info = mybir.SyncInfo(on_wait=[w], on_update=[])
                        out.append(nop)
                    inst.sync_info = mybir.SyncInfo(
                        on_wait=[waits[-1]], on_update=list(si.on_update)
                    )
                out.append(inst)
            insts[:] = out
    return nc


def _build_nc(trace_scopes=False):
    nc = bass.Bass()
    f32 = mybir.dt.float32
    bf16 = mybir.dt.bfloat16

    xt_in = nc.declare_dram_parameter("xt", [C, T], bf16, isOutput=False)
    wqk_in = nc.declare_dram_parameter("wqk", [C, 128], bf16, isOutput=False)
    wv_in = nc.declare_dram_parameter("wv", [C, HS], bf16, isOutput=False)
    wp_in = nc.declare_dram_parameter("wp", [HS, C], bf16, isOutput=False)
    css_in = nc.declare_dram_parameter("css", [128, 2 * T], bf16, isOutput=False)
    out_u = nc.declare_dram_parameter("out_u", [T, C], bf16, isOutput=True)
    cs_out = nc.declare_dram_parameter("cs", [1, T], bf16, isOutput=True)

    Exp = mybir.ActivationFunctionType.Exp
    f8 = mybir.dt.float8e4
    DR = mybir.MatmulPerfMode.DoubleRow
    SWAP_MASK = [i ^ 1 for i in range(32)]

    with tile.TileContext(nc) as tc:
        with (
            tc.tile_pool(name="big", bufs=1) as big,
            tc.tile_pool(name="ropet", bufs=3) as ropet,
            tc.tile_pool(name="ptp", bufs=5) as ptp,
            tc.tile_pool(name="ytsb", bufs=2) as ytsb,
            tc.tile_pool(name="outp", bufs=3) as outp,
            tc.tile_pool(name="ps", bufs=1, space="PSUM") as ps,
        ):
            # ---- resident inputs ----
            # weights first (small, needed immediately), then xt/cc/ss chunked
            # per t-block in consumption order so proj(0) can start after ~1MB.
            wqk_sb = big.tile([128, 4, 128], bf16)
            nc.scalar.dma_start(out=wqk_sb, in_=wqk_in.ap().rearrange("(n p) m -> p n m", p=128))
            wv_sb = big.tile([128, 4, HS], bf16)
            nc.scalar.dma_start(out=wv_sb, in_=wv_in.ap().rearrange("(n p) m -> p n m", p=128))
            wp_sb = big.tile([HS, C], bf16)
            nc.scalar.dma_start(out=wp_sb, in_=wp_in.ap())
            xt_sb = big.tile([128, 4, T], bf16)
            _xt_r = xt_in.ap().rearrange("(n p) t -> p n t", p=128)
            css_sb = big.tile([128, 2, T], bf16)
            _css_r = css_in.ap().rearrange("p (n t) -> p n t", n=2)
            # block-0 data on the scalar queue too (ACT is idle in the ramp)
            nc.scalar.dma_start(out=xt_sb[:, :, 0:T // 8], in_=_xt_r[:, :, 0:T // 8])
            nc.scalar.dma_start(out=css_sb[:, :, 0:T // 8], in_=_css_r[:, :, 0:T // 8])
            for _c in range(1, 8):
                _t0 = _c * (T // 8)
                nc.sync.dma_start(out=xt_sb[:, :, _t0:_t0 + T // 8],
                                  in_=_xt_r[:, :, _t0:_t0 + T // 8])
                nc.sync.dma_start(out=css_sb[:, :, _t0:_t0 + T // 8],
                                  in_=_css_r[:, :, _t0:_t0 + T // 8])
            cc_sb = css_sb[:, 0, :]
            ss_sb = css_sb[:, 1, :]

            qkr = big.tile([128, T], bf16)    # rows 0:64 = q_rot^T, 64:128 = k_rot^T
            krqr = big.tile([128, T], bf16)   # rows 0:64 = k_rot^T, 64:128 = q_rot^T
            v_ones = big.tile([128, NJC, HS + 1], bf16)
            nc.vector.memset(v_ones[:, :, HS], 1.0)

            # ---- per-block pieces ----
            def proj_qk(tb):
                tc0 = tb * TB
                qk_ps = ps.tile([128, TB], f32, tag="qk", bufs=1)
                for cn in range(4):
                    nc.tensor.matmul(qk_ps, wqk_sb[:, cn, :], xt_sb[:, cn, tc0:tc0 + TB],
                                     start=(cn == 0), stop=(cn == 3))
                # pair-swapped duplicate of (q|k) for the rope cross terms
                # (StreamShuffle requires same src/dst dtype -> keep f32)
                qks_f = ropet.tile([128, TB], f32, tag="qks")
                nc.vector.stream_shuffle(qks_f, qk_ps, SWAP_MASK)
                t2 = ropet.tile([128, TB], bf16, tag="rt")
                nc.vector.tensor_mul(t2, qk_ps, cc_sb[:, tc0:tc0 + TB])
                t1 = ropet.tile([128, TB], bf16, tag="rt")
                nc.vector.tensor_mul(t1, qks_f, ss_sb[:, tc0:tc0 + TB])
                nc.vector.tensor_add(qkr[:, tc0:tc0 + TB], t2, t1)
            def krqr_copy(tb):
                # swapped-halves duplicate for the row-paired S^T matmuls.
                # Dispatched from the scalar queue, positioned AFTER the
                # previous block's exps so the wait on rope(tb) never stalls
                # the exp stream (and never sits behind input DMA dispatches).
                tc0 = tb * TB
                nc.scalar.dma_start(out=krqr[0:64, tc0:tc0 + TB], in_=qkr[64:128, tc0:tc0 + TB])
                nc.scalar.dma_start(out=krqr[64:128, tc0:tc0 + TB], in_=qkr[0:64, tc0:tc0 + TB])

            def proj_v(tb):
                tc0 = tb * TB
                # v in (t, d) layout; all 4 t-chunks share one PSUM bank
                v_ps = ps.tile([128, 4, HS], f32, tag="aux", bufs=2)
                for t4 in range(4):
                    p0 = tc0 + t4 * 128
                    for cn in range(4):
                        nc.tensor.matmul(v_ps[:, t4, :], xt_sb[:, cn, p0:p0 + 128],
                                         wv_sb[:, cn, :],
                                         start=(cn == 0), stop=(cn == 3))
                nc.vector.tensor_copy(v_ones[:, tb * 4:(tb + 1) * 4, 0:HS], v_ps)

            def attn_block(ib):
                i0 = ib * TB
                nj = 4 * ib + 4
                yt_ps = ps.tile([128, TB], f32, tag="yt", bufs=1)
                pend = []  # (pt-slice, j) waiting for their PV matmul

                def flush_pv(n):
                    while len(pend) > n:
                        pt_, j_ = pend.pop(0)
                        v0_ = max(0, j_ * JC - i0)
                        nc.tensor.matmul(yt_ps[0:HS + 1, v0_:TB], v_ones[:, j_, :],
                                         pt_[:, v0_:TB],
                                         start=(j_ == 0), stop=(j_ == nj - 1),
                                         skip_group_check=True)

                for m in range(nj // 2):
                    j_e, j_o = 2 * m, 2 * m + 1
                    ve = max(0, j_e * JC - i0)
                    vo = max(0, j_o * JC - i0)
                    st2 = ps.tile([128, 2, TB], f32, tag="st", bufs=2)
                    nc.tensor.matmul(st2[:, 0, ve:TB], krqr[0:64, j_e * JC:(j_e + 1) * JC],
                                     qkr[0:64, i0 + ve:i0 + TB], tile_position=(0, 0))
                    nc.tensor.matmul(st2[:, 1, vo:TB], qkr[64:128, j_o * JC:(j_o + 1) * JC],
                                     krqr[64:128, i0 + vo:i0 + TB], tile_position=(64, 0))
                    diag = j_o * JC + JC - 1 > i0  # pair touches the diagonal band
                    pt2 = ptp.tile([128, 2, TB], bf16, tag="pt")
                    if not diag:
                        nc.scalar.activation(pt2, st2, Exp, scale=0.125)  # FD=1024
                    else:
                        nc.scalar.activation(pt2[:, 0, ve:TB], st2[:, 0, ve:TB],
                                             Exp, scale=0.125)
                        nc.scalar.activation(pt2[:, 1, vo:TB], st2[:, 1, vo:TB],
                                             Exp, scale=0.125)
                        for half, j in ((0, j_e), (1, j_o)):
                            if j * JC + JC - 1 > i0:
                                b1 = min(TB, max(0, j * JC - i0) + JC)
                                nc.gpsimd.affine_select(
                                    out=pt2[:, half, 0:b1], in_=pt2[:, half, 0:b1],
                                    compare_op=mybir.AluOpType.is_ge,
                                    fill=0.0, base=i0 - j * JC,
                                    pattern=[[1, b1]], channel_multiplier=-1)
                    pend.append((pt2[:, 0, :], j_e))
                    pend.append((pt2[:, 1, :], j_o))
                    flush_pv(2)  # keep 1 pair in flight so PE never waits on ACT
                flush_pv(0)

                # single cast: y rows 0:64 + denominator row 64
                yt_sb = ytsb.tile([HS + 1, TB], bf16, tag="yts")
                nc.vector.tensor_copy(yt_sb, yt_ps[0:HS + 1, :])
                nc.sync.dma_start(out=cs_out.ap()[0:1, i0:i0 + TB],
                                  in_=yt_sb[HS:HS + 1, :])
                return yt_sb

            def cproj_block(ib, yt_sb):
                i0 = ib * TB
                for q2 in range(2):
                    ot2 = outp.tile([128, 2, TB], bf16, tag="ot")
                    for h in range(2):
                        q = 2 * q2 + h
                        op_ps = ps.tile([128, TB], f32, tag="aux", bufs=2)
                        nc.tensor.matmul(op_ps, yt_sb[0:HS, q * 128:(q + 1) * 128],
                                         wp_sb)
                        nc.vector.tensor_copy(ot2[:, h, :], op_ps)
                    nc.sync.dma_start(
                        out=out_u.ap()[i0 + q2 * 256:i0 + (q2 + 1) * 256, :].rearrange(
                            "(n p) t -> p n t", p=128),
                        in_=ot2)

            # ---- software pipeline: proj one block ahead of attention,
            # ---- c_proj deferred one more block (hides PSUM-bank reuse).
            # tile_wait_until phases the units in the scheduler's simulated
            # timeline; without it the ready-list scheduler hoists ALL
            # projection matmuls ahead of attention and ACT idles ~30us.
            UNIT_MS = 0.008  # simulated time per pipeline unit (order-only)
            yts = {}
            with tc.tile_wait_until(0 * UNIT_MS):
                proj_qk(0)
                krqr_copy(0)
                proj_v(0)
            for b in range(1, NTB):
                with tc.tile_wait_until(b * UNIT_MS):
                    proj_qk(b)
                    yts[b - 1] = attn_block(b - 1)
                    krqr_copy(b)
                    if b >= 2:
                        cproj_block(b - 2, yts.pop(b - 2))
                    proj_v(b)
            with tc.tile_wait_until(NTB * UNIT_MS):
                yts[NTB - 1] = attn_block(NTB - 1)
                cproj_block(NTB - 2, yts.pop(NTB - 2))
                cproj_block(NTB - 1, yts.pop(NTB - 1))

    _legalize_waits(nc)
    return nc


_cached = {}


def _get_nc():
    if "nc" not in _cached:
        _cached["nc"] = _build_nc()
    return _cached["nc"]


def _prep_inputs(x, rope, W_attn, W_proj):
    bf16 = ml_dtypes.bfloat16
    xt = np.ascontiguousarray(x[0].T).astype(bf16)          # (C, T)
    cos = np.asarray(rope[..., 0], dtype=np.float32)        # (T, HS//2)
    sin = np.asarray(rope[..., 1], dtype=np.float32)
    cc64 = np.repeat(cos.T, 2, axis=0)                      # (HS, T)
    ss64 = np.repeat(sin.T, 2, axis=0)
    ss64[0::2, :] *= -1.0                                   # sign folded: even rows -sin
    cc = np.concatenate([cc64, cc64], axis=0).astype(np.float32)
    ss = np.concatenate([ss64, ss64], axis=0).astype(np.float32)
    css = np.ascontiguousarray(np.stack([cc, ss], axis=1).reshape(128, -1)).astype(bf16)

    Wa = np.asarray(W_attn, dtype=np.float32)
    Wp = np.asarray(W_proj, dtype=np.float32)

    in_maps = []
    for h in range(NCORES):
        Wq = Wa[h * HS:(h + 1) * HS]                        # (HS, C)
        Wk = Wa[C + h * HS:C + (h + 1) * HS]
        Wv = Wa[2 * C + h * HS:2 * C + (h + 1) * HS]
        wqk = np.concatenate([Wq.T, Wk.T], axis=1).astype(bf16)        # (C, 128)
        wv = np.ascontiguousarray(Wv.T).astype(bf16)                   # (C, HS)
        wp = np.ascontiguousarray(Wp[:, h * HS:(h + 1) * HS].T).astype(bf16)  # (HS, C)
        in_maps.append({
            "xt": xt, "wqk": wqk, "wv": wv, "wp": wp, "css": css,
        })
    return in_maps


def run_cores(x, rope, W_attn, W_proj, trace=False):
    """Returns (list of per-core result dicts, BassKernelResults)."""
    nc = _get_nc()
    in_maps = _prep_inputs(x, rope, W_attn, W_proj)
    res = run_bass_kernel_spmd(nc, in_maps, list(range(NCORES)), trace=trace)
    return res


def kernel(x, rope, mask, W_attn, W_proj):
    res = run_cores(x, rope, W_attn, W_proj, trace=False)
    out = np.zeros((T, C), dtype=np.float32)
    for h in range(NCORES):
        r = res.results[h]
        cs = np.asarray(r["cs"], dtype=np.float32).reshape(T, 1)
        out += np.asarray(r["out_u"], dtype=np.float32) / cs
    return out.reshape(B, T, C).astype(np.float32)


# revision 18
# speedup vs baseline: 1.0533x; 1.0533x over previous
"""Causal self-attention with RoPE on 8 TRN2 NeuronCores.

Sharding: tensor-parallel over heads (H=8 -> 1 head per core).
v2: per-block proj/attn software pipeline (keeps PE warm), merged FD=1024
exp ops on ACT, rope pair-swap via DVE stream_shuffle (instead of a second
PE projection), bf16 cos/sin tables, merged v-cast and yt+denominator cast.

Each core computes, for its head h:
    q,k projections (bf16 matmuls, fp32 PSUM) -> pair-swap via stream_shuffle
    -> RoPE on DVE (bf16 tables)  -> v projection
    S^T blocks (j,i) via K=64 row-paired concurrent matmuls into 2-bank tiles
    P^T = exp(S^T/8) on ACT (bf16 out, FD=1024 for full pairs), causal
    masking via gpsimd affine_select (covers prefix + diagonal band)
    y_u^T = [v | ones]^T-weighted PV matmuls  (row 64 = softmax denominator)
    out_u = y_u @ Wp_h^T on-device; host computes sum_h out_u_h / colsum_h.
"""
import sys

sys.path.insert(0, "/opt/trn_rl_repo")

import numpy as np
import ml_dtypes

import concourse.bass as bass
import concourse.mybir as mybir
import concourse.tile as tile
from concourse.bass_utils import run_bass_kernel_spmd

B, T, C, H = 1, 4096, 512, 8
HS = C // H  # 64
NCORES = 8
TB = 512           # t-block width for projections / i-block width for attention
NTB = T // TB      # 8
JC = 128           # j-chunk width
NJC = T // JC      # 32

_ctr = [0]


def _legalize_waits(nc):
    """This walrus build accepts at most one sem-wait command per hw
    instruction; move extra waits onto same-engine NoOps inserted before."""
    for f in nc.m.functions:
        for bb in f.blocks:
            insts = bb.instructions
            out = []
            for inst in insts:
                si = inst.sync_info
                if si is not None and len(si.on_wait) > 1:
                    waits = list(si.on_wait)
                    for w in waits[:-1]:
                        _ctr[0] += 1
                        nop = mybir.InstNoOp(name=f"I-waitsplit-{_ctr[0]}")
                        nop.engine = inst.engine
                        nop.sync_info = mybir.SyncInfo(on_wait=[w], on_update=[])
                        out.append(nop)
                    inst.sync_info = mybir.SyncInfo(
                        on_wait=[waits[-1]], on_update=list(si.on_update)
                    )
                out.append(inst)
            insts[:] = out
    return nc


def _build_nc(trace_scopes=False):
    nc = bass.Bass()
    f32 = mybir.dt.float32
    bf16 = mybir.dt.bfloat16

    xt_in = nc.declare_dram_parameter("xt", [C, T], bf16, isOutput=False)
    wqk_in = nc.declare_dram_parameter("wqk", [C, 128], bf16, isOutput=False)
    wv_in = nc.declare_dram_parameter("wv", [C, HS], bf16, isOutput=False)
    wp_in = nc.declare_dram_parameter("wp", [HS, C], bf16, isOutput=False)
    cc_in = nc.declare_dram_parameter("cc", [128, T], bf16, isOutput=False)
    ss_in = nc.declare_dram_parameter("ss", [128, T], bf16, isOutput=False)
    out_u = nc.declare_dram_parameter("out_u", [T, C], bf16, isOutput=True)
    cs_out = nc.declare_dram_parameter("cs", [1, T], bf16, isOutput=True)

    Exp = mybir.ActivationFunctionType.Exp
    f8 = mybir.dt.float8e4
    DR = mybir.MatmulPerfMode.DoubleRow
    SWAP_MASK = [i ^ 1 for i in range(32)]

    with tile.TileContext(nc) as tc:
        with (
            tc.tile_pool(name="big", bufs=1) as big,
            tc.tile_pool(name="ropet", bufs=3) as ropet,
            tc.tile_pool(name="ptp", bufs=5) as ptp,
            tc.tile_pool(name="ytsb", bufs=2) as ytsb,
            tc.tile_pool(name="outp", bufs=3) as outp,
            tc.tile_pool(name="ps", bufs=1, space="PSUM") as ps,
        ):
            # ---- resident inputs ----
            # weights first (small, needed immediately), then xt/cc/ss chunked
            # per t-block in consumption order so proj(0) can start after ~1MB.
            wqk_sb = big.tile([128, 4, 128], bf16)
            nc.sync.dma_start(out=wqk_sb, in_=wqk_in.ap().rearrange("(n p) m -> p n m", p=128))
            wv_sb = big.tile([128, 4, HS], bf16)
            nc.sync.dma_start(out=wv_sb, in_=wv_in.ap().rearrange("(n p) m -> p n m", p=128))
            wp_sb = big.tile([HS, C], bf16)
            nc.sync.dma_start(out=wp_sb, in_=wp_in.ap())
            xt_sb = big.tile([128, 4, T], bf16)
            _xt_r = xt_in.ap().rearrange("(n p) t -> p n t", p=128)
            cc_sb = big.tile([128, T], bf16)
            ss_sb = big.tile([128, T], bf16)
            for _c in range(8):
                _t0 = _c * (T // 8)
                nc.sync.dma_start(out=xt_sb[:, :, _t0:_t0 + T // 8],
                                  in_=_xt_r[:, :, _t0:_t0 + T // 8])
                nc.sync.dma_start(out=cc_sb[:, _t0:_t0 + T // 8], in_=cc_in.ap()[:, _t0:_t0 + T // 8])
                nc.sync.dma_start(out=ss_sb[:, _t0:_t0 + T // 8], in_=ss_in.ap()[:, _t0:_t0 + T // 8])

            qkr = big.tile([128, T], bf16)    # rows 0:64 = q_rot^T, 64:128 = k_rot^T
            krqr = big.tile([128, T], bf16)   # rows 0:64 = k_rot^T, 64:128 = q_rot^T
            v_ones = big.tile([128, NJC, HS + 1], bf16)
            nc.vector.memset(v_ones[:, :, HS], 1.0)

            # ---- per-block pieces ----
            def proj_block(tb):
                tc0 = tb * TB
                qk_ps = ps.tile([128, TB], f32, tag="qk", bufs=1)
                for cn in range(4):
                    nc.tensor.matmul(qk_ps, wqk_sb[:, cn, :], xt_sb[:, cn, tc0:tc0 + TB],
                                     start=(cn == 0), stop=(cn == 3))
                # pair-swapped duplicate of (q|k) for the rope cross terms
                # (StreamShuffle requires same src/dst dtype -> keep f32)
                qks_f = ropet.tile([128, TB], f32, tag="qks")
                nc.vector.stream_shuffle(qks_f, qk_ps, SWAP_MASK)
                t2 = ropet.tile([128, TB], bf16, tag="rt")
                nc.vector.tensor_mul(t2, qk_ps, cc_sb[:, tc0:tc0 + TB])
                t1 = ropet.tile([128, TB], bf16, tag="rt")
                nc.vector.tensor_mul(t1, qks_f, ss_sb[:, tc0:tc0 + TB])
                nc.vector.tensor_add(qkr[:, tc0:tc0 + TB], t2, t1)
                # swapped-halves duplicate for the row-paired S^T matmuls
                nc.sync.dma_start(out=krqr[0:64, tc0:tc0 + TB], in_=qkr[64:128, tc0:tc0 + TB])
                nc.sync.dma_start(out=krqr[64:128, tc0:tc0 + TB], in_=qkr[0:64, tc0:tc0 + TB])
                # v in (t, d) layout; all 4 t-chunks share one PSUM bank
                v_ps = ps.tile([128, 4, HS], f32, tag="aux", bufs=2)
                for t4 in range(4):
                    p0 = tc0 + t4 * 128
                    for cn in range(4):
                        nc.tensor.matmul(v_ps[:, t4, :], xt_sb[:, cn, p0:p0 + 128],
                                         wv_sb[:, cn, :],
                                         start=(cn == 0), stop=(cn == 3))
                nc.vector.tensor_copy(v_ones[:, tb * 4:(tb + 1) * 4, 0:HS], v_ps)

            def attn_block(ib):
                i0 = ib * TB
                nj = 4 * ib + 4
                yt_ps = ps.tile([128, TB], f32, tag="yt", bufs=1)
                pend = []  # (pt-slice, j) waiting for their PV matmul

                def flush_pv(n):
                    while len(pend) > n:
                        pt_, j_ = pend.pop(0)
                        v0_ = max(0, j_ * JC - i0)
                        nc.tensor.matmul(yt_ps[0:HS + 1, v0_:TB], v_ones[:, j_, :],
                                         pt_[:, v0_:TB],
                                         start=(j_ == 0), stop=(j_ == nj - 1),
                                         skip_group_check=True)

                for m in range(nj // 2):
                    j_e, j_o = 2 * m, 2 * m + 1
                    ve = max(0, j_e * JC - i0)
                    vo = max(0, j_o * JC - i0)
                    st2 = ps.tile([128, 2, TB], f32, tag="st", bufs=2)
                    nc.tensor.matmul(st2[:, 0, ve:TB], krqr[0:64, j_e * JC:(j_e + 1) * JC],
                                     qkr[0:64, i0 + ve:i0 + TB], tile_position=(0, 0))
                    nc.tensor.matmul(st2[:, 1, vo:TB], qkr[64:128, j_o * JC:(j_o + 1) * JC],
                                     krqr[64:128, i0 + vo:i0 + TB], tile_position=(64, 0))
                    diag = j_o * JC + JC - 1 > i0  # pair touches the diagonal band
                    pt2 = ptp.tile([128, 2, TB], bf16, tag="pt")
                    if not diag:
                        nc.scalar.activation(pt2, st2, Exp, scale=0.125)  # FD=1024
                    else:
                        nc.scalar.activation(pt2[:, 0, ve:TB], st2[:, 0, ve:TB],
                                             Exp, scale=0.125)
                        nc.scalar.activation(pt2[:, 1, vo:TB], st2[:, 1, vo:TB],
                                             Exp, scale=0.125)
                        for half, j in ((0, j_e), (1, j_o)):
                            if j * JC + JC - 1 > i0:
                                b1 = min(TB, max(0, j * JC - i0) + JC)
                                nc.gpsimd.affine_select(
                                    out=pt2[:, half, 0:b1], in_=pt2[:, half, 0:b1],
                                    compare_op=mybir.AluOpType.is_ge,
                                    fill=0.0, base=i0 - j * JC,
                                    pattern=[[1, b1]], channel_multiplier=-1)
                    pend.append((pt2[:, 0, :], j_e))
                    pend.append((pt2[:, 1, :], j_o))
                    flush_pv(2)  # keep 1 pair in flight so PE never waits on ACT
                flush_pv(0)

                # single cast: y rows 0:64 + denominator row 64
                yt_sb = ytsb.tile([HS + 1, TB], bf16, tag="yts")
                nc.vector.tensor_copy(yt_sb, yt_ps[0:HS + 1, :])
                nc.sync.dma_start(out=cs_out.ap()[0:1, i0:i0 + TB],
                                  in_=yt_sb[HS:HS + 1, :])
                return yt_sb

            def cproj_block(ib, yt_sb):
                i0 = ib * TB
                for q2 in range(2):
                    ot2 = outp.tile([128, 2, TB], bf16, tag="ot")
                    for h in range(2):
                        q = 2 * q2 + h
                        op_ps = ps.tile([128, TB], f32, tag="aux", bufs=2)
                        nc.tensor.matmul(op_ps, yt_sb[0:HS, q * 128:(q + 1) * 128],
                                         wp_sb)
                        nc.vector.tensor_copy(ot2[:, h, :], op_ps)
                    nc.sync.dma_start(
                        out=out_u.ap()[i0 + q2 * 256:i0 + (q2 + 1) * 256, :].rearrange(
                            "(n p) t -> p n t", p=128),
                        in_=ot2)

            # ---- software pipeline: proj one block ahead of attention,
            # ---- c_proj deferred one more block (hides PSUM-bank reuse).
            # tile_wait_until phases the units in the scheduler's simulated
            # timeline; without it the ready-list scheduler hoists ALL
            # projection matmuls ahead of attention and ACT idles ~30us.
            UNIT_MS = 0.008  # simulated time per pipeline unit (order-only)
            yts = {}
            with tc.tile_wait_until(0 * UNIT_MS):
                proj_block(0)
            for b in range(1, NTB):
                with tc.tile_wait_until(b * UNIT_MS):
                    proj_block(b)
                    yts[b - 1] = attn_block(b - 1)
                    if b >= 2:
                        cproj_block(b - 2, yts.pop(b - 2))
            with tc.tile_wait_until(NTB * UNIT_MS):
                yts[NTB - 1] = attn_block(NTB - 1)
                cproj_block(NTB - 2, yts.pop(NTB - 2))
                cproj_block(NTB - 1, yts.pop(NTB - 1))

    _legalize_waits(nc)
    return nc


_cached = {}


def _get_nc():
    if "nc" not in _cached:
        _cached["nc"] = _build_nc()
    return _cached["nc"]


def _prep_inputs(x, rope, W_attn, W_proj):
    bf16 = ml_dtypes.bfloat16
    xt = np.ascontiguousarray(x[0].T).astype(bf16)          # (C, T)
    cos = np.asarray(rope[..., 0], dtype=np.float32)        # (T, HS//2)
    sin = np.asarray(rope[..., 1], dtype=np.float32)
    cc64 = np.repeat(cos.T, 2, axis=0)                      # (HS, T)
    ss64 = np.repeat(sin.T, 2, axis=0)
    ss64[0::2, :] *= -1.0                                   # sign folded: even rows -sin
    cc = np.ascontiguousarray(np.concatenate([cc64, cc64], axis=0)).astype(bf16)
    ss = np.ascontiguousarray(np.concatenate([ss64, ss64], axis=0)).astype(bf16)

    Wa = np.asarray(W_attn, dtype=np.float32)
    Wp = np.asarray(W_proj, dtype=np.float32)

    in_maps = []
    for h in range(NCORES):
        Wq = Wa[h * HS:(h + 1) * HS]                        # (HS, C)
        Wk = Wa[C + h * HS:C + (h + 1) * HS]
        Wv = Wa[2 * C + h * HS:2 * C + (h + 1) * HS]
        wqk = np.concatenate([Wq.T, Wk.T], axis=1).astype(bf16)        # (C, 128)
        wv = np.ascontiguousarray(Wv.T).astype(bf16)                   # (C, HS)
        wp = np.ascontiguousarray(Wp[:, h * HS:(h + 1) * HS].T).astype(bf16)  # (HS, C)
        in_maps.append({
            "xt": xt, "wqk": wqk, "wv": wv, "wp": wp, "cc": cc, "ss": ss,
        })
    return in_maps


def run_cores(x, rope, W_attn, W_proj, trace=False):
    """Returns (list of per-core result dicts, BassKernelResults)."""
    nc = _get_nc()
    in_maps = _prep_inputs(x, rope, W_attn, W_proj)
    res = run_bass_kernel_spmd(nc, in_maps, list(range(NCORES)), trace=trace)
    return res


def kernel(x, rope, mask, W_attn, W_proj):
    res = run_cores(x, rope, W_attn, W_proj, trace=False)
    out = np.zeros((T, C), dtype=np.float32)
    for h in range(NCORES):
        r = res.results[h]
        cs = np.asarray(r["cs"], dtype=np.float32).reshape(T, 1)
        out += np.asarray(r["out_u"], dtype=np.float32) / cs
    return out.reshape(B, T, C).astype(np.float32)
